# revision 2
# baseline (speedup 1.0000x reference)
"""BitTransformerLayer on 8 Trainium2 NeuronCores — v3.

v3 over v2:
  - Attention operands in BF16 (qk, et, vaug, ocat, wo): FWL weight loads,
    halved SBUF/DMA. Denominators stay fp32/fp32r. exp split per query-half
    for a tighter PE/ScalarE pipeline (psS bufs=3).
  - FFN1/FFN2 in fp8e4 DoubleRow (contraction 256/pass): ternary weights are
    exact in fp8; activation codes quantized straight to the fp8 grid
    (codes = f8(bf16(y*127/max)) — no integer rounding pass needed; the
    deviation from round() is within the fp8 quantization noise class).
    Pairing d = cc*256 + k*128 + p matches host weight packing; device side
    it's just a [:, 2cc:2cc+2, :] slice of the per-token [128, C, 128] fp8
    transposed-code tensors.
  - x1 residual fp32, in place over x (no extra region, no bf16 loss).
  - Per-token pipeline: out_proj epilogue -> rmsnorm2+quant (G) -> FFN1 ->
    quant2 -> FFN2; quant scale factors fold rstd (grid is scale-invariant).
"""
import sys

for _p in ("/opt/trn_rl_repo", "/opt/pypackages"):
    if _p not in sys.path:
        sys.path.append(_p)

import numpy as np
import concourse.bass as bass
import concourse.tile as tile
from concourse import bacc, mybir
from concourse.bass_utils import run_bass_kernel_spmd
from concourse.masks import make_identity

FP32 = mybir.dt.float32
FP32R = mybir.dt.float32r
BF16 = mybir.dt.bfloat16
FP8 = mybir.dt.float8e4

B, S, D, H, FF = 8, 1024, 1024, 16, 4096
DH = D // H
T = S // 128
C = D // 128
FC = FF // 128
QH = S // 512
EPS = 1e-6
DR = mybir.MatmulPerfMode.DoubleRow

Act = mybir.ActivationFunctionType
Alu = mybir.AluOpType

_last_results = None


def _build(w1s: float, w2s: float, flags: dict):
    nc = bacc.Bacc()

    x_d = nc.declare_dram_parameter("x", [S, D], FP32, isOutput=False)
    wqkvT_d = nc.declare_dram_parameter("wqkvT", [D, 3 * D], FP32R, isOutput=False)
    woT_d = nc.declare_dram_parameter("woT", [D, D], BF16, isOutput=False)
    w1f8_d = nc.declare_dram_parameter("w1f8", [128, 8 * FF], FP8, isOutput=False)
    w2f8_d = nc.declare_dram_parameter("w2f8", [128, 32 * D], FP8, isOutput=False)
    extras = {}
    for nm, shp, fl in (("bqkv", [3 * D], "bqkv"), ("bo", [D], "bo"),
                        ("b1", [FF], "b1"), ("b2", [D], "b2"), ("n2w", [D], "n2w")):
        if flags[fl]:
            extras[nm] = nc.declare_dram_parameter(nm, shp, FP32, isOutput=False)
    out_d = nc.declare_dram_parameter("out", [S, D], FP32, isOutput=True)

    # ---- SBUF arena ----
    A0 = 16512
    R0 = A0                      # 32K: xnT | ocat(16K)+et0(16K) | h_db | scrG/ot
    R1 = A0 + 32 * 1024          # 32K: qk bf16 | w1sb8 fp8
    R2 = A0 + 64 * 1024          # 32K: x fp32 -> x1 in place
    R3 = A0 + 96 * 1024          # 16.6K: vaug bf16 | yqT8 fp8
    R4 = R3 + 16640              # 32K: stg (E) | hqT8 fp8
    ARENA_END = R4 + 32 * 1024
    nc.sbuf_base = ARENA_END

    man = nc.alloc_sbuf_tensor_at
    xnT = [man(f"xnT{c}", [128, S], FP32R, offset=R0 + c * 4096) for c in range(C)]
    ocat = [man(f"ocn{c}", [128, S], BF16, offset=R0 + c * 2048) for c in range(C)]
    h_db = [man(f"h_{i}", [128, FF], BF16, offset=R0 + i * 8192) for i in range(2)]
    scrG = man("scrG", [128, D], FP32, offset=R0 + 28672)
    ot_sb = [man(f"ot{t}", [128, D], FP32, offset=R0 + t * 4096) for t in range(T)]

    qk = [man(f"qk{f}", [128, S], BF16, offset=R1 + f * 2048) for f in range(16)]
    w1sb8 = man("w1sb8", [128, 4, 2, FF], FP8, offset=R1)

    x_sb = [man(f"x_{t}", [128, D], FP32, offset=R2 + t * 4096) for t in range(T)]

    vaug = [man(f"va{t}", [128, H, DH + 1], BF16, offset=R3 + t * 2080)
            for t in range(T)]
    yqT8 = [man(f"yqT8_{t}", [128, C, 128], FP8, offset=R3 + t * 1024)
            for t in range(T)]

    stg_sb = [man(f"stg{i}", [65, 512], FP32R, offset=R4 + i * 2048)
              for i in range(3)]
    hqT8 = [man(f"hqT8_{t}", [128, FC, 128], FP8, offset=R4 + t * 4096)
            for t in range(T)]

    def bcast_row(dram_ap, lo, n, width, pool, tag, parts=128):
        t_ = pool.tile([parts, width], FP32, tag=tag, name=tag)
        ap = bass.AP(tensor=dram_ap.tensor, offset=dram_ap.offset + lo,
                     ap=[[width, n], [0, parts // n], [1, width]])
        nc.sync.dma_start(out=t_, in_=ap)
        return t_

    with tile.TileContext(nc) as tc:
        small_cm = tc.tile_pool(name="small", bufs=1)
        small = small_cm.__enter__()

        eps_t = small.tile([128, 1], FP32, tag="eps", name="eps")
        nc.vector.memset(eps_t, EPS)
        ident = small.tile([128, 128], FP32, tag="ident", name="ident")
        make_identity(nc, ident)
        ones_f = small.tile([128, 64], FP32, tag="ones_f", name="ones_f")
        nc.vector.memset(ones_f, 1.0)
        ones_r = small.tile([128, 64], FP32R, tag="ones_r", name="ones_r")
        nc.vector.tensor_copy(out=ones_r, in_=ones_f)
        sfac = [small.tile([128, 1], FP32, tag=f"sfac{t}", name=f"sfac{t}")
                for t in range(T)]
        gfac = [small.tile([128, 1], FP32, tag=f"gfac{t}", name=f"gfac{t}")
                for t in range(T)]

        # ============ Stage A ============
        pxn_cm = tc.tile_pool(name="pxn", bufs=2)
        pxn = pxn_cm.__enter__()
        psScr_cm = tc.tile_pool(name="psScr", bufs=2, space="PSUM")
        psScr = psScr_cm.__enter__()
        psA_cm = tc.tile_pool(name="psA", bufs=2, space="PSUM")
        psA = psA_cm.__enter__()

        for t in range(T):
            x_t = x_sb[t]
            nc.sync.dma_start(out=x_t[:], in_=x_d[t * 128:(t + 1) * 128, :])
            scr = psScr.tile([128, D], FP32, tag="sqscr", name="sqscr")
            ssq = pxn.tile([128, 1], FP32, tag="ssq", name="ssq")
            nc.scalar.activation(scr, x_t[:], Act.Square, accum_out=ssq)
            rstd = pxn.tile([128, 1], FP32, tag="rstd", name="rstd")
            nc.scalar.activation(rstd, ssq, Act.Sqrt, bias=eps_t, scale=1.0 / D)
            nc.vector.reciprocal(rstd, rstd)
            xn_t = pxn.tile([128, D], FP32, tag="xn", name="xn")
            nc.vector.tensor_scalar_mul(out=xn_t, in0=x_t[:], scalar1=rstd)
            tp = psA.tile([128, D], FP32, tag="tp", name="tp")
            for c in range(C):
                nc.tensor.transpose(tp[:, c * 128:(c + 1) * 128],
                                    xn_t[:, c * 128:(c + 1) * 128], ident)
            for c in range(C):
                dst = xnT[c][:, t * 128:(t + 1) * 128]
                src = tp[:, c * 128:(c + 1) * 128]
                if c % 2 == 0:
                    nc.vector.tensor_copy(out=dst, in_=src)
                else:
                    nc.scalar.activation(dst, src, Act.Copy)
        psA_cm.__exit__(None, None, None)
        psScr_cm.__exit__(None, None, None)
        pxn_cm.__exit__(None, None, None)

        # ============ Stage D: QKV ============
        pwq_cm = tc.tile_pool(name="pwq", bufs=3)
        pwq = pwq_cm.__enter__()

        def _qk_epilogue(f, ps_pair):
            if flags["bqkv"]:
                bq_f = small.tile([128, 1], FP32, tag=f"bq{f}", name=f"bq{f}")
                nc.sync.dma_start(
                    out=bq_f,
                    in_=extras["bqkv"][f * 128:(f + 1) * 128].rearrange(
                        "(p o) -> p o", o=1))
                for n in range(QH):
                    tmpb = pwq.tile([128, 512], FP32, tag="tmpb", name="tmpb")
                    nc.vector.tensor_scalar_add(out=tmpb, in0=ps_pair[n],
                                                scalar1=bq_f)
                    nc.vector.tensor_copy(out=qk[f][:, n * 512:(n + 1) * 512],
                                          in_=tmpb)
            else:
                for n in range(QH):
                    dst = qk[f][:, n * 512:(n + 1) * 512]
                    if (f + n) % 2 == 0:
                        nc.vector.tensor_copy(out=dst, in_=ps_pair[n])
                    else:
                        nc.scalar.activation(dst, ps_pair[n], Act.Copy)

        # V first: its vector-heavy epilogue then hides under the Q/K stream,
        # so the PE never idles at the D->E boundary (HAM clock stays warm).
        psV_cm = tc.tile_pool(name="psV", bufs=1, space="PSUM")
        psV = psV_cm.__enter__()
        ones16 = small.tile([128, H, 1], FP32, tag="ones16", name="ones16")
        nc.vector.memset(ones16, 1.0)
        for t in range(T):
            nc.vector.tensor_copy(out=vaug[t][:, :, DH:DH + 1], in_=ones16)
        for vh in range(2):
            v_ps = [psV.tile([128, 512], FP32, tag=f"vps{t}", name=f"vps{t}")
                    for t in range(T)]
            for c in range(C):
                wv = pwq.tile([128, 512], FP32R, tag="wv", name="wv")
                nc.sync.dma_start(
                    out=wv,
                    in_=wqkvT_d[c * 128:(c + 1) * 128,
                                2 * D + vh * 512: 2 * D + (vh + 1) * 512])
                for t in range(T):
                    nc.tensor.matmul(v_ps[t], lhsT=xnT[c][:, t * 128:(t + 1) * 128],
                                     rhs=wv, start=(c == 0), stop=(c == C - 1))
            for t in range(T):
                src = v_ps[t].rearrange("p (hh dd) -> p hh dd", dd=DH)
                dst = vaug[t][:, vh * 8:(vh + 1) * 8, 0:DH]
                if flags["bqkv"]:
                    bvb = bcast_row(extras["bqkv"][:], 2 * D + vh * 512, 1, 512,
                                    pwq, "bvb")
                    tmpv = pwq.tile([128, 512], FP32, tag="tmpv", name="tmpv")
                    nc.vector.tensor_add(
                        out=tmpv.rearrange("p (hh dd) -> p hh dd", dd=DH),
                        in0=src,
                        in1=bvb.rearrange("p (hh dd) -> p hh dd", dd=DH))
                    nc.vector.tensor_copy(
                        out=dst,
                        in_=tmpv.rearrange("p (hh dd) -> p hh dd", dd=DH))
                else:
                    nc.vector.tensor_copy(out=dst, in_=src)
        psV_cm.__exit__(None, None, None)

        psD_cm = tc.tile_pool(name="psD", bufs=1, space="PSUM")
        psD = psD_cm.__enter__()
        for fg in range(4):
            qk_ps = [[psD.tile([128, 512], FP32, tag=f"qkps{fi}_{n}",
                               name=f"qkps{fi}_{n}") for n in range(QH)]
                     for fi in range(4)]
            for c in range(C):
                wq4 = pwq.tile([128, 512], FP32R, tag="wq4", name="wq4")
                nc.sync.dma_start(
                    out=wq4,
                    in_=wqkvT_d[c * 128:(c + 1) * 128, fg * 512:(fg + 1) * 512])
                for fi in range(4):
                    for n in range(QH):
                        nc.tensor.matmul(qk_ps[fi][n],
                                         lhsT=wq4[:, fi * 128:(fi + 1) * 128],
                                         rhs=xnT[c][:, n * 512:(n + 1) * 512],
                                         start=(c == 0), stop=(c == C - 1))
            for fi in range(4):
                _qk_epilogue(fg * 4 + fi, qk_ps[fi])
        psD_cm.__exit__(None, None, None)
        pwq_cm.__exit__(None, None, None)

        # ============ Stage E: attention ============
        pet_cm = tc.tile_pool(name="pet", bufs=2)
        pet = pet_cm.__enter__()
        pod_cm = tc.tile_pool(name="pod", bufs=2)
        pod = pod_cm.__enter__()
        psS_cm = tc.tile_pool(name="psS", bufs=2, space="PSUM")
        psS = psS_cm.__enter__()
        psO_cm = tc.tile_pool(name="psO", bufs=2, space="PSUM")
        psO = psO_cm.__enter__()

        stg_cnt = 0
        for h in range(H):
            ft = h // 2
            bq = (h % 2) * 64
            o_pss = [psO.tile([DH + 1, 512], FP32, tag=f"ops{qh}", name=f"ops{qh}")
                     for qh in range(QH)]
            ets = [None] * T
            last_sps = None
            for kt in range(T):
                s_ps = psS.tile([128, S], FP32, tag="sps", name="sps")
                last_sps = s_ps
                for qh in range(QH):
                    nc.tensor.matmul(
                        s_ps[:, qh * 512:(qh + 1) * 512],
                        lhsT=qk[8 + ft][bq:bq + 64, kt * 128:(kt + 1) * 128],
                        rhs=qk[ft][bq:bq + 64, qh * 512:(qh + 1) * 512],
                        start=True, stop=True)
                et = pet.tile([128, S], BF16, tag="et", name="et")
                nc.scalar.activation(et, s_ps, Act.Exp,
                                     scale=float(1.0 / np.sqrt(DH)))
                ets[kt] = et
                if kt > 0:
                    for qh in range(QH):
                        nc.tensor.matmul(o_pss[qh], lhsT=vaug[kt - 1][:, h, :],
                                         rhs=ets[kt - 1][:, qh * 512:(qh + 1) * 512],
                                         start=(kt - 1 == 0), stop=False)
            for qh in range(QH):
                nc.tensor.matmul(o_pss[qh], lhsT=vaug[T - 1][:, h, :],
                                 rhs=ets[T - 1][:, qh * 512:(qh + 1) * 512],
                                 start=False, stop=True)
            c = h // 2
            for qh in range(QH):
                stg = stg_sb[stg_cnt % 3]
                stg_cnt += 1
                nc.vector.tensor_copy(out=stg[:], in_=o_pss[qh])
                bc = last_sps[0:64, qh * 512:(qh + 1) * 512]
                nc.tensor.matmul(bc, lhsT=ones_r[64:65, :], rhs=stg[64:65, :],
                                 start=True, stop=True)
                nc.vector.reciprocal_approx_fast(out=bc, in_=bc)
                if h % 2 == 0:
                    nc.vector.tensor_mul(
                        out=ocat[c][0:64, qh * 512:(qh + 1) * 512],
                        in0=stg[0:64, :], in1=bc)
                else:
                    onor = pod.tile([64, 512], BF16, tag="onor", name="onor")
                    nc.vector.tensor_mul(out=onor, in0=stg[0:64, :], in1=bc)
                    nc.gpsimd.dma_start(
                        out=ocat[c][64:128, qh * 512:(qh + 1) * 512], in_=onor)
        psO_cm.__exit__(None, None, None)
        psS_cm.__exit__(None, None, None)
        pod_cm.__exit__(None, None, None)
        pet_cm.__exit__(None, None, None)

        # ============ Stage F + G inlined ============
        pg_cm = tc.tile_pool(name="pg", bufs=2)
        pg = pg_cm.__enter__()
        pwo_cm = tc.tile_pool(name="pwo", bufs=1)
        pwo = pwo_cm.__enter__()
        pyq_cm = tc.tile_pool(name="pyq", bufs=2)
        pyq = pyq_cm.__enter__()
        psF_cm = tc.tile_pool(name="psF", bufs=1, space="PSUM")
        psF = psF_cm.__enter__()

        # wo loads FIRST on the sync queue (needed at F start), then the w1
        # prefetch (needed only at H; its qk-region fences would otherwise
        # block wo behind the tail of attention).
        wo_sb = []
        for c in range(C):
            w = pwo.tile([128, D], BF16, tag=f"wo{c}", name=f"wo{c}")
            nc.sync.dma_start(out=w, in_=woT_d[c * 128:(c + 1) * 128, :])
            wo_sb.append(w)
        for i in range(4):
            nc.sync.dma_start(
                out=w1sb8[:, i, :, :],
                in_=w1f8_d[:, i * 2 * FF:(i + 1) * 2 * FF].rearrange(
                    "p (k f) -> p k f", k=2))

        n2wb = None
        if flags["n2w"]:
            n2wb = bcast_row(extras["n2w"][:], 0, 1, D, small, "n2wb")
        b1b = []
        if flags["b1"]:
            for fh in range(FF // 512):
                b1b.append(bcast_row(extras["b1"][:], fh * 512, 1, 512,
                                     pg, f"b1b{fh}"))

        def stage_g(t):
            # x1 == x_sb[t] (in place). Quant grid is rmsnorm-invariant.
            src = x_sb[t][:]
            if n2wb is not None:
                xw = pg.tile([128, D], FP32, tag="xw", name="xw")
                nc.vector.tensor_mul(out=xw, in0=src, in1=n2wb)
                src = xw[:]
            ssq = pg.tile([128, 1], FP32, tag="ssq2", name="ssq2")
            nc.scalar.activation(scrG[:], src, Act.Square, accum_out=ssq)
            rstd = pg.tile([128, 1], FP32, tag="rstd2", name="rstd2")
            nc.scalar.activation(rstd, ssq, Act.Sqrt, bias=eps_t, scale=1.0 / D)
            nc.vector.reciprocal(rstd, rstd)
            m_t = pg.tile([128, 1], FP32, tag="mt", name="mt")
            nc.vector.tensor_reduce(out=m_t, in_=src, axis=mybir.AxisListType.X,
                                    op=Alu.max, apply_absolute_value=True)
            nc.vector.tensor_scalar_max(out=m_t, in0=m_t, scalar1=1e-5)
            s_t = pg.tile([128, 1], FP32, tag="st", name="st")
            nc.vector.reciprocal(s_t, m_t)
            nc.vector.tensor_scalar_mul(out=s_t, in0=s_t, scalar1=127.0)
            nc.vector.tensor_scalar(out=sfac[t], in0=m_t, scalar1=rstd,
                                    scalar2=float(w1s / 127.0),
                                    op0=Alu.mult, op1=Alu.mult)
            yqbf = pyq.tile([128, D], BF16, tag="yqbf", name="yqbf")
            nc.vector.tensor_scalar_mul(out=yqbf, in0=src, scalar1=s_t)
            yqTb = pyq.tile([128, C, 128], BF16, tag="yqTb", name="yqTb")
            nc.sync.dma_start_transpose(yqTb[:, :, :], yqbf)
            nc.vector.tensor_copy(out=yqT8[t][:, :, :], in_=yqTb)

        bob = None
        if flags["bo"]:
            bob = bcast_row(extras["bo"][:], 0, 1, D, pwo, "bob")
        for t in range(T):
            x1_ps = psF.tile([128, D], FP32, tag=f"x1ps{t % 3}",
                             name=f"x1ps{t % 3}")
            for c in range(C):
                for oh in range(2):
                    nc.tensor.matmul(x1_ps[:, oh * 512:(oh + 1) * 512],
                                     lhsT=ocat[c][:, t * 128:(t + 1) * 128],
                                     rhs=wo_sb[c][:, oh * 512:(oh + 1) * 512],
                                     start=(c == 0), stop=(c == C - 1))
            dst = x_sb[t][:]
            nc.vector.tensor_add(out=dst, in0=x1_ps, in1=dst)
            if bob is not None:
                nc.vector.tensor_add(out=dst, in0=dst, in1=bob)
            stage_g(t)
        psF_cm.__exit__(None, None, None)
        pyq_cm.__exit__(None, None, None)
        pwo_cm.__exit__(None, None, None)

        # ============ Stage H: FFN1 (fp8 DoubleRow) + quant2 ============
        pw2_cm = tc.tile_pool(name="pw2", bufs=3)
        pw2 = pw2_cm.__enter__()
        phq_cm = tc.tile_pool(name="phq", bufs=2)
        phq = phq_cm.__enter__()
        psH_cm = tc.tile_pool(name="psH", bufs=1, space="PSUM")
        psH = psH_cm.__enter__()

        w2_pre = {}
        for cc in range(2):
            w2t = pw2.tile([128, 2, D], FP8, tag="w2", name="w2")
            nc.sync.dma_start(
                out=w2t,
                in_=w2f8_d[:, cc * 2 * D:(cc + 1) * 2 * D].rearrange(
                    "p (k n) -> p k n", k=2))
            w2_pre[cc] = w2t

        for t in range(T):
            h_t = h_db[t % 2]
            for half in range(2):
                hp = psH.tile([128, 2048], FP32, tag=f"hp{half}", name=f"hp{half}")
                for cc in range(4):
                    for fh in range(4):
                        nc.tensor.matmul(
                            hp[:, fh * 512:(fh + 1) * 512],
                            lhsT=yqT8[t][:, 2 * cc:2 * cc + 2, :],
                            rhs=w1sb8[:, cc, :, half * 2048 + fh * 512:
                                      half * 2048 + (fh + 1) * 512],
                            start=(cc == 0), stop=(cc == 3),
                            perf_mode=DR)
                for fh in range(4):
                    hslice = h_t[:, half * 2048 + fh * 512:
                                 half * 2048 + (fh + 1) * 512]
                    pslice = hp[:, fh * 512:(fh + 1) * 512]
                    if flags["b1"]:
                        tmp = pg.tile([128, 512], FP32, tag="b1tmp", name="b1tmp")
                        nc.vector.tensor_scalar_mul(out=tmp, in0=pslice,
                                                    scalar1=sfac[t])
                        nc.vector.tensor_add(out=tmp, in0=tmp,
                                             in1=b1b[half * 4 + fh])
                        nc.scalar.activation(hslice, tmp, Act.Gelu)
                    else:
                        nc.scalar.activation(hslice, pslice, Act.Gelu,
                                             scale=sfac[t])
            m2 = pg.tile([128, 1], FP32, tag="m2", name="m2")
            nc.vector.tensor_reduce(out=m2, in_=h_t[:], axis=mybir.AxisListType.X,
                                    op=Alu.max, apply_absolute_value=True)
            nc.vector.tensor_scalar_max(out=m2, in0=m2, scalar1=1e-5)
            s2 = pg.tile([128, 1], FP32, tag="s2", name="s2")
            nc.vector.reciprocal(s2, m2)
            nc.vector.tensor_scalar_mul(out=s2, in0=s2, scalar1=127.0)
            nc.vector.tensor_scalar_mul(out=gfac[t], in0=m2,
                                        scalar1=float(w2s / 127.0))
            hqbf = phq.tile([128, FF], BF16, tag="hqbf", name="hqbf")
            nc.vector.tensor_scalar_mul(out=hqbf, in0=h_t[:], scalar1=s2)
            hqTb = phq.tile([128, FC, 128], BF16, tag="hqTb", name="hqTb")
            nc.sync.dma_start_transpose(hqTb[:, :, :], hqbf)
            nc.scalar.activation(hqT8[t][:, 0:16, :], hqTb[:, 0:16, :], Act.Copy)
            nc.vector.tensor_copy(out=hqT8[t][:, 16:32, :], in_=hqTb[:, 16:32, :])
        psH_cm.__exit__(None, None, None)
        phq_cm.__exit__(None, None, None)

        # ============ Stage I: FFN2 (fp8 DoubleRow) + residual -> out ========
        psI_cm = tc.tile_pool(name="psI", bufs=1, space="PSUM")
        psI = psI_cm.__enter__()
        for tg in range(2):
            ts = range(tg * 4, tg * 4 + 4)
            o2_ps = {t: psI.tile([128, D], FP32, tag=f"o2ps{t % 4}",
                                 name=f"o2ps{t % 4}") for t in ts}
            for cc in range(16):
                if tg == 0 and cc in w2_pre:
                    w2t = w2_pre.pop(cc)
                else:
                    w2t = pw2.tile([128, 2, D], FP8, tag="w2", name="w2")
                    nc.sync.dma_start(
                        out=w2t,
                        in_=w2f8_d[:, cc * 2 * D:(cc + 1) * 2 * D].rearrange(
                            "p (k n) -> p k n", k=2))
                for t in ts:
                    for oh in range(2):
                        nc.tensor.matmul(
                            o2_ps[t][:, oh * 512:(oh + 1) * 512],
                            lhsT=hqT8[t][:, 2 * cc:2 * cc + 2, :],
                            rhs=w2t[:, :, oh * 512:(oh + 1) * 512],
                            start=(cc == 0), stop=(cc == 15),
                            perf_mode=DR)
            b2b = None
            if flags["b2"]:
                b2b = bcast_row(extras["b2"][:], 0, 1, D, pw2, "b2b")
            for t in ts:
                nc.vector.scalar_tensor_tensor(
                    out=ot_sb[t][:], in0=o2_ps[t], scalar=gfac[t],
                    in1=x_sb[t][:], op0=Alu.mult, op1=Alu.add)
                if b2b is not None:
                    nc.vector.tensor_add(out=ot_sb[t][:], in0=ot_sb[t][:], in1=b2b)
                nc.sync.dma_start(out=out_d[t * 128:(t + 1) * 128, :],
                                  in_=ot_sb[t][:])
        psI_cm.__exit__(None, None, None)
        pw2_cm.__exit__(None, None, None)
        pg_cm.__exit__(None, None, None)
        small_cm.__exit__(None, None, None)

    nc.finalize()
    return nc


def kernel(**inputs):
    global _last_results
    x = np.ascontiguousarray(np.asarray(inputs["x"], dtype=np.float32))
    n1 = np.asarray(inputs["norm1_w"], dtype=np.float32)
    n2 = np.asarray(inputs["norm2_w"], dtype=np.float32)
    wqkv = np.asarray(inputs["in_proj_w"], dtype=np.float32)
    bqkv = np.asarray(inputs["in_proj_b"], dtype=np.float32)
    wo = np.asarray(inputs["out_proj_w"], dtype=np.float32)
    bo = np.asarray(inputs["out_proj_b"], dtype=np.float32)
    w1 = np.asarray(inputs["w1"], dtype=np.float32)
    b1 = np.asarray(inputs["b1"], dtype=np.float32)
    w2 = np.asarray(inputs["w2"], dtype=np.float32)
    b2 = np.asarray(inputs["b2"], dtype=np.float32)

    import ml_dtypes
    import os

    wqkvT = np.ascontiguousarray((wqkv * n1[None, :]).T.astype(np.float32))
    woT = np.ascontiguousarray(wo.T).astype(ml_dtypes.bfloat16)

    def ternarize(w):
        s = np.float32(1.0) / np.clip(np.abs(w).mean(dtype=np.float32),
                                      np.float32(1e-5), None)
        q = np.clip(np.round(w * s), -1.0, 1.0).astype(np.float32)
        return q, float(np.float32(1.0) / s)

    w1q, w1s = ternarize(w1)
    w2q, w2s = ternarize(w2)
    # DoubleRow pair packing: contraction element (p, k) of chunk cc maps to
    # input-dim cc*256 + k*128 + p, matching the device-side [:, 2cc:2cc+2, :]
    # slices of the [128, C, 128] transposed code tensors.
    w1f8 = np.ascontiguousarray(
        w1q.T.reshape(4, 2, 128, FF).transpose(2, 0, 1, 3).reshape(128, 8 * FF)
    ).astype(ml_dtypes.float8_e4m3)
    w2f8 = np.ascontiguousarray(
        w2q.T.reshape(16, 2, 128, D).transpose(2, 0, 1, 3).reshape(128, 32 * D)
    ).astype(ml_dtypes.float8_e4m3)

    flags = {
        "bqkv": bool(np.any(bqkv != 0)),
        "bo": bool(np.any(bo != 0)),
        "b1": bool(np.any(b1 != 0)),
        "b2": bool(np.any(b2 != 0)),
        "n2w": not bool(np.all(n2 == 1.0)),
    }

    nc = _build(w1s, w2s, flags)

    shared = dict(wqkvT=wqkvT, woT=woT, w1f8=w1f8, w2f8=w2f8)
    for nm, arr in (("bqkv", bqkv), ("bo", bo), ("b1", b1), ("b2", b2),
                    ("n2w", n2)):
        if flags[nm]:
            shared[nm] = arr

    in_maps = [dict(x=np.ascontiguousarray(x[b]), **shared) for b in range(B)]
    res = run_bass_kernel_spmd(nc, in_maps, list(range(B)))
    _last_results = res
    return np.stack([res.results[b]["out"] for b in range(B)]).astype(np.float32)
